# revision 39
# baseline (speedup 1.0000x reference)
"""Trainium2 Bass kernel for nn_BandSplitDCTFilter.

Math: the reference's mirror-FFT DCT / band filter / inverse collapses to
    out_c = C1 (Z_c) C2^T - S1 (Z_c) S2^T,   Z_c = (A x_c A^T) .* W_eff_c
with A[k,j] = 2cos(pi k (2j+1)/128); C2/S2 carry the irfft half-spectrum
weights u_l and the 1/(4HW) scale; W_eff = pad(W_low)+pad(W_mid)+W_high
merges the three bands.  Then y = x_out @ proj_w^T and LayerNorm.

Sharding: pure data-parallel, one sample per core (B=8 = 8 cores).

v12: DMA-engine time is ~45ns per descriptor regardless of 256B/512B
size, so the two layout pivots are descriptor-count-bound and must be
overlapped, not just shrunk:
  - P1 (k<->w): strided stores stream per 1024-col chunk DURING the row
    DCT (4 small stores per pipe); loads are 2 big contiguous reads.
  - P2 (n<->k): stores are 2 big contiguous reads of U2s; the strided
    (gather) side moved to the LOADS, issued per 16-column quarter so
    they hide under the inverse-k/proj/LN tail of the previous quarter.
  - paired PSUM tiles [128,1024] halve drain instruction count; the
    W_eff multiply runs on gpsimd (SBUF in-place) to unload DVE.
  - LN stats via batched bn_stats on [128,2,256] PSUM pairs + manual
    two-half combine; normalize on gpsimd; y stores per quarter.
All data bf16 except PSUM/stats; host up-casts y.
"""

import os

os.environ.setdefault("JAX_PLATFORMS", "axon,cpu")

import numpy as np
import ml_dtypes

import bass_rust
import concourse.bass as bass
import concourse.mybir as mybir
from concourse.tile import TileContext, ScopedClock
from concourse.bass_utils import run_bass_kernel_spmd

# ---------------------------------------------------------------------------
# Workarounds: this container's walrus rejects >1 sync wait per instruction.
# ---------------------------------------------------------------------------

_wait_ctr = 0


def _split_multi_waits(nc, max_waits=1):
    global _wait_ctr
    for f in nc.m.functions:
        for bb in f.blocks:
            out = []
            dirty = False
            for ins in bb.instructions:
                si = ins.sync_info
                if si is not None and len(si.on_wait) > max_waits:
                    waits = list(si.on_wait)
                    for w in waits[:-max_waits]:
                        _wait_ctr += 1
                        nop = bass_rust.InstNoOp(name=f"I-waitsplit-{_wait_ctr}")
                        nop.engine = ins.engine
                        nop.sync_info = mybir.SyncInfo(on_wait=[w], on_update=[])
                        out.append(nop)
                    ins.sync_info = mybir.SyncInfo(
                        on_wait=waits[-max_waits:], on_update=list(si.on_update)
                    )
                    dirty = True
                out.append(ins)
            if dirty:
                bb.instructions = out


def _patched_drain_and_barrier(self, tick_clock, wait_clock):
    nc = self.nc
    probe = nc.sync.nop(nofuse=True)
    wait_clock.add_sem_waits(probe.ins, ScopedClock({None: tick_clock.global_clock}))
    si = probe.ins.sync_info
    waits = list(si.on_wait) if si is not None else []
    probe.ins.sync_info = mybir.SyncInfo(on_wait=waits[:1], on_update=[])
    name2sem = {s.name: s for s in self.sems.allocated().values()}
    for w in waits[1:]:
        nc.sync.nop(nofuse=True)._wait_ge(name2sem[w.ant_name], w.wait_value)
    nc.sync.drain()
    nc.all_engine_barrier()
    popped = nc._tile_sem_poison_stack.pop()
    assert popped is self._sem_poison
    nc.clear_and_free_semaphores(list(self.sems.allocated().values()))
    nc.all_engine_barrier()


TileContext._drain_and_barrier = _patched_drain_and_barrier

# ---------------------------------------------------------------------------

B, H, W, C = 8, 64, 64, 256
N = H * W
F32 = mybir.dt.float32
BF16 = mybir.dt.bfloat16
ALU = mybir.AluOpType
ACTF = mybir.ActivationFunctionType


def _host_matrices():
    k = np.arange(64)
    j = np.arange(64)
    ang = np.pi * k[:, None] * (2 * j[None, :] + 1) / 128.0
    A = 2.0 * np.cos(ang)
    u = np.where(k == 0, 1.0, 2.0)
    C1T = np.cos(ang)
    S1T = np.sin(ang)
    C2T = u[:, None] * np.cos(ang) / 16384.0
    S2T = u[:, None] * np.sin(ang) / 16384.0

    AT = A.T.astype(np.float32)                                   # [h, k]
    khbd = np.zeros((128, 128), np.float32)
    khbd[0:64, 0:64] = AT
    khbd[64:128, 64:128] = AT
    cs2_half = np.concatenate([C2T, S2T], axis=1)                 # [l, 128]
    cs2 = np.concatenate([cs2_half, cs2_half], axis=0)
    ICS = np.concatenate([C1T, -S1T], axis=0)
    return (khbd.astype(ml_dtypes.bfloat16),
            cs2.astype(ml_dtypes.bfloat16),
            np.ascontiguousarray(ICS.astype(ml_dtypes.bfloat16)))


_NC_CACHE = {}


def _build_nc(apply_gb):
    nc = bass.Bass(trn_type="TRN2")

    xa_d = nc.dram_tensor("xra", [128, 4096], BF16, kind="ExternalInput")
    xb_d = nc.dram_tensor("xrb", [128, 4096], BF16, kind="ExternalInput")
    kh_d = nc.dram_tensor("kh", [128, 128], BF16, kind="ExternalInput")
    cs_d = nc.dram_tensor("cs", [128, 128], BF16, kind="ExternalInput")
    ics_d = nc.dram_tensor("ics", [128, 64], BF16, kind="ExternalInput")
    w_d = nc.dram_tensor("weff", [128, 8192], BF16, kind="ExternalInput")
    pjt_d = nc.dram_tensor("pjt", [128, 512], BF16, kind="ExternalInput")
    gb_d = nc.dram_tensor("gb", [2, 256], F32, kind="ExternalInput")
    y_d = nc.dram_tensor("y", [128, 8192], BF16, kind="ExternalOutput")

    with TileContext(nc) as tc:
        with (
            tc.tile_pool(name="consts", bufs=1) as consts,
            tc.tile_pool(name="wf", bufs=1) as wf,
            tc.tile_pool(name="pAx", bufs=1) as pAx,
            tc.tile_pool(name="pBx", bufs=1) as pBx,
            tc.tile_pool(name="pAt", bufs=1) as pAt,
            tc.tile_pool(name="pBt", bufs=1) as pBt,
            tc.tile_pool(name="pT2", bufs=1) as pT2,
            tc.tile_pool(name="pZ", bufs=1) as pZ,
            tc.tile_pool(name="pU", bufs=1) as pU,
            tc.tile_pool(name="pUs", bufs=1) as pUs,
            tc.tile_pool(name="pY", bufs=1) as pY,
            tc.tile_pool(name="dramp", bufs=1, space="DRAM") as dramp,
            tc.tile_pool(name="ps", bufs=2, space="PSUM") as ps,
            tc.tile_pool(name="psy", bufs=4, space="PSUM") as psy,
            tc.tile_pool(name="small", bufs=16) as small,
        ):
            # ---- constants (gpsimd queue; x loads get sync/scalar) ----
            khbd = consts.tile([128, 128], BF16, tag="khbd")
            cs2 = consts.tile([128, 128], BF16, tag="cs2")
            ics = consts.tile([128, 64], BF16, tag="ics")
            pjt = consts.tile([128, 512], BF16, tag="pjt")
            nc.gpsimd.dma_start(out=khbd[:], in_=kh_d[:])
            nc.gpsimd.dma_start(out=cs2[:], in_=cs_d[:])
            nc.gpsimd.dma_start(out=ics[:], in_=ics_d[:])
            nc.gpsimd.dma_start(out=pjt[:], in_=pjt_d[:])
            eps = consts.tile([128, 1], F32, tag="eps")
            nc.vector.memset(eps[:], 1e-5)
            weff = wf.tile([128, 8192], BF16, tag="wf")
            nc.gpsimd.dma_start(out=weff[:], in_=w_d[:])
            if apply_gb:
                gt = consts.tile([128, 256], F32, tag="gt")
                bt = consts.tile([128, 256], F32, tag="bt")
                gb_ap = gb_d.ap()
                g_b = bass.AP(tensor=gb_ap.tensor, offset=0, ap=[[0, 128], [1, 256]])
                b_b = bass.AP(tensor=gb_ap.tensor, offset=256, ap=[[0, 128], [1, 256]])
                nc.gpsimd.dma_start(out=gt[:], in_=g_b)
                nc.gpsimd.dma_start(out=bt[:], in_=b_b)

            # ---- load x (w-split pipes: A = w 0:32, B = w 32:64) ----
            XA = pAx.tile([128, 4096], BF16, tag="pAx", name="XA")
            XB = pBx.tile([128, 4096], BF16, tag="pBx", name="XB")
            nc.sync.dma_start(out=XA[:, 0:1024], in_=xa_d[:, 0:1024])
            nc.sync.dma_start(out=XA[:, 1024:4096], in_=xa_d[:, 1024:4096])
            nc.scalar.dma_start(out=XB[:, 0:1024], in_=xb_d[:, 0:1024])
            nc.scalar.dma_start(out=XB[:, 1024:4096], in_=xb_d[:, 1024:4096])

            # D1[k, (w64,c256)] row = k; contiguous stores, strided loads.
            # Split per k-half so load hulls only touch their own stores.
            D1h = [dramp.tile([32, 16384], BF16, tag="d1", name=f"D1_{kh}")
                   for kh in range(2)]
            D1ht = [t[:].tensor for t in D1h]

            # ---- S2; P1 stores are contiguous blocks per (pipe,wq) ----
            # T1[(wq,k),(w16,c256)]; D1[k, w*256+c] with w = 32P+16wq+w16.
            def s2_p1(Xt, T1t, P, io):
                for jp in range(4):
                    sl = slice(jp * 1024, (jp + 1) * 1024)
                    pt = ps.tile([128, 1024], F32, tag="ps")
                    nc.tensor.matmul(pt[:, 0:512], khbd[:],
                                     Xt[:, jp * 1024:jp * 1024 + 512],
                                     start=True, stop=True)
                    nc.tensor.matmul(pt[:, 512:1024], khbd[:],
                                     Xt[:, jp * 1024 + 512:(jp + 1) * 1024],
                                     start=True, stop=True)
                    eng = nc.vector.tensor_copy if jp % 2 == 0 else nc.scalar.copy
                    eng(T1t[:, sl], pt[:])
                for kh in range(2):
                    for wq in range(2):
                        cofs = (32 * P + 16 * wq) * 256
                        io.dma_start(
                            out=D1h[kh][:, cofs:cofs + 4096],
                            in_=T1t[wq * 64 + kh * 32:wq * 64 + (kh + 1) * 32, :])

            T1A = pAt.tile([128, 4096], BF16, tag="pAt", name="T1A")
            T1B = pBt.tile([128, 4096], BF16, tag="pBt", name="T1B")
            s2_p1(XA, T1A, 0, nc.sync)
            s2_p1(XB, T1B, 1, nc.scalar)

            # ---- P1 loads: strided gather per (kg, k-quarter), streamed ----
            # T2[(kg,w64),(k32,c256)] <- D1[row kg*32+kq*8+k8, col w*256+c]
            T2 = pT2.tile([128, 8192], BF16, tag="pT2", name="T2")
            for kq in range(4):
                for kg in range(2):
                    kh, krem = divmod(kg * 32 + kq * 8, 32)
                    src = bass.AP(
                        tensor=D1ht[kh],
                        offset=krem * 16384,
                        ap=[[256, 64], [16384, 8], [1, 256]],
                    )
                    io = nc.sync if kg == 0 else nc.scalar
                    io.dma_start(
                        out=T2[kg * 64:(kg + 1) * 64,
                               kq * 2048:(kq + 1) * 2048],
                        in_=src)

            # ---- S4: col DCT; W_eff multiply on gpsimd in SBUF ----
            Zp = pZ.tile([128, 8192], BF16, tag="pZ", name="Zp")
            for jp in range(8):
                sl = slice(jp * 1024, (jp + 1) * 1024)
                pt = ps.tile([128, 1024], F32, tag="ps")
                nc.tensor.matmul(pt[:, 0:512], khbd[:],
                                 T2[:, jp * 1024:jp * 1024 + 512],
                                 start=True, stop=True)
                nc.tensor.matmul(pt[:, 512:1024], khbd[:],
                                 T2[:, jp * 1024 + 512:(jp + 1) * 1024],
                                 start=True, stop=True)
                nc.vector.tensor_mul(Zp[:, sl], pt[:], weff[:, sl])

            # ---- S5 + contiguous P2 stores per kg ----
            U2s = pU.tile([128, 16384], BF16, tag="pU", name="U2s")
            D2h = [dramp.tile([128, 8192], BF16, tag="d2", name=f"D2_{kg}")
                   for kg in range(2)]
            D2ht = [t[:].tensor for t in D2h]
            for kg in range(2):
                off = kg * 64
                for jp in range(8):
                    base = kg * 8192 + jp * 1024
                    pt = ps.tile([128, 1024], F32, tag="ps")
                    for h2 in range(2):
                        zsl = slice(jp * 1024 + h2 * 512, jp * 1024 + (h2 + 1) * 512)
                        nc.tensor.matmul(pt[:, h2 * 512:(h2 + 1) * 512],
                                         cs2[off:off + 64, :],
                                         Zp[off:off + 64, zsl],
                                         start=True, stop=True)
                    eng = nc.vector.tensor_copy if jp % 4 == 0 else nc.scalar.copy
                    eng(U2s[:, base:base + 1024], pt[:])
                    # stream the contiguous store for this drain right away
                    # (2KB runs, ~128 descriptors) so P2's store half rides
                    # under S5 compute instead of serializing after it
                    io = nc.sync if jp % 2 == 0 else nc.scalar
                    io.dma_start(out=D2h[kg][:, jp * 1024:(jp + 1) * 1024],
                                 in_=U2s[:, base:base + 1024])

            # ---- tail: per n-quarter: gather pivot, inverse-k, proj, LN ----
            Ustk = pUs.tile([128, 16384], BF16, tag="pUs", name="Ustk")
            X01 = [
                pAx.tile([128, 4096], BF16, tag="pAx", name="X01_0"),
                pBx.tile([128, 4096], BF16, tag="pBx", name="X01_1"),
            ]
            Yraw = pY.tile([128, 8192], BF16, tag="pY", name="Yraw")

            statsall = small.tile([128, 192], F32, tag="statsall")
            sv = statsall[:].rearrange("p (t s) -> p t s", s=6)
            msum = small.tile([128, 32], F32, tag="msum")
            mdif = small.tile([128, 32], F32, tag="mdif")
            cvs = small.tile([128, 32], F32, tag="cvs")
            varv = small.tile([128, 32], F32, tag="varv")
            rstd = small.tile([128, 32], F32, tag="rstd")
            nmr = small.tile([128, 32], F32, tag="nmr")


            # gather loads: Ustk[(cs,kg,k32), (n16,c256)]; kg0 on sync (can
            # start right after the kg0 store, under S5 kg1), kg1 on scalar.
            # All quarters enqueue up front and stream ahead of the tail.
            for q in range(4):
                for cs in range(2):
                    for kg in range(2):
                        src = bass.AP(
                            tensor=D2ht[kg],
                            offset=(cs * 64 + q * 16) * 8192,
                            ap=[[256, 32], [8192, 16], [1, 256]],
                        )
                        io = nc.scalar if kg == 0 else nc.sync
                        r0 = cs * 64 + kg * 32
                        io.dma_start(
                            out=Ustk[r0:r0 + 32, q * 4096:(q + 1) * 4096],
                            in_=src)

            for q in range(4):
                for chalf in range(2):
                    pt = ps.tile([128, 1024], F32, tag="ps")
                    for gi, g in enumerate((2 * q, 2 * q + 1)):
                        for nn in range(8):
                            n0 = 8 * g + nn
                            cofs = n0 * 256 + chalf * 128
                            nc.tensor.matmul(
                                pt[:, gi * 512 + nn * 64:gi * 512 + (nn + 1) * 64],
                                Ustk[:, cofs:cofs + 128],
                                ics[:], start=True, stop=True,
                            )
                    eng = nc.vector.tensor_copy if chalf == 0 else nc.scalar.copy
                    eng(X01[chalf][:, 2 * q * 512:(2 * q + 2) * 512], pt[:])
                ptys = []
                for tp in range(4 * q, 4 * q + 4):       # pairs of proj tiles
                    pty = psy.tile([128, 512], F32, tag="psy")
                    for half in range(2):
                        t2 = 2 * tp + half
                        co = slice(half * 256, (half + 1) * 256)
                        nc.tensor.matmul(pty[:, co],
                                         X01[0][:, t2 * 128:(t2 + 1) * 128],
                                         pjt[:, 0:256], start=True, stop=False)
                        nc.tensor.matmul(pty[:, co],
                                         X01[1][:, t2 * 128:(t2 + 1) * 128],
                                         pjt[:, 256:512], start=False, stop=True)
                    for half in range(2):
                        t2 = 2 * tp + half
                        nc.vector.bn_stats(
                            out=statsall[:, t2 * 6:(t2 + 1) * 6],
                            in_=pty[:, half * 256:(half + 1) * 256])
                    ptys.append(pty)
                # LN stats for this quarter's 8 tiles:
                # mean = (m_e+m_o)/2 ; var = (cv_e+cv_o)/256 + ((m_e-m_o)/2)^2
                gs = slice(q * 8, q * 8 + 8)
                nc.gpsimd.tensor_tensor(out=msum[:, gs], in0=sv[:, gs, 1],
                                        in1=sv[:, gs, 4], op=ALU.add)
                nc.gpsimd.tensor_tensor(out=mdif[:, gs], in0=sv[:, gs, 1],
                                        in1=sv[:, gs, 4], op=ALU.subtract)
                nc.gpsimd.tensor_tensor(out=cvs[:, gs], in0=sv[:, gs, 2],
                                        in1=sv[:, gs, 5], op=ALU.add)
                nc.gpsimd.tensor_tensor(out=mdif[:, gs], in0=mdif[:, gs],
                                        in1=mdif[:, gs], op=ALU.mult)
                nc.gpsimd.tensor_scalar_mul(cvs[:, gs], cvs[:, gs], 1.0 / 256.0)
                nc.vector.scalar_tensor_tensor(
                    out=varv[:, gs], in0=mdif[:, gs], scalar=0.25,
                    in1=cvs[:, gs], op0=ALU.mult, op1=ALU.add)
                nc.scalar.activation(out=rstd[:, gs], in_=varv[:, gs],
                                     func=ACTF.Sqrt, bias=eps[:], scale=1.0)
                nc.vector.reciprocal(rstd[:, gs], rstd[:, gs])
                nc.vector.scalar_tensor_tensor(
                    out=nmr[:, gs], in0=msum[:, gs], scalar=-0.5,
                    in1=rstd[:, gs], op0=ALU.mult, op1=ALU.mult)
                # normalize fused into the PSUM drain (per-tile scale/bias)
                for i, tp in enumerate(range(4 * q, 4 * q + 4)):
                    pty = ptys[i]
                    for half in range(2):
                        t2 = 2 * tp + half
                        ysl = slice(t2 * 256, (t2 + 1) * 256)
                        src = pty[:, half * 256:(half + 1) * 256]
                        if t2 % 2 == 0:
                            nc.vector.tensor_scalar(
                                out=Yraw[:, ysl], in0=src,
                                scalar1=rstd[:, t2:t2 + 1],
                                scalar2=nmr[:, t2:t2 + 1],
                                op0=ALU.mult, op1=ALU.add,
                            )
                        else:
                            nc.scalar.activation(
                                out=Yraw[:, ysl], in_=src,
                                func=ACTF.Identity,
                                bias=nmr[:, t2:t2 + 1],
                                scale=rstd[:, t2:t2 + 1],
                            )
                        if apply_gb:
                            nc.vector.tensor_mul(Yraw[:, ysl], Yraw[:, ysl],
                                                 gt[:])
                            nc.gpsimd.tensor_add(Yraw[:, ysl], Yraw[:, ysl],
                                                 bt[:])
                nc.gpsimd.dma_start(out=y_d[:, q * 2048:(q + 1) * 2048],
                                    in_=Yraw[:, q * 2048:(q + 1) * 2048])

    _split_multi_waits(nc)
    return nc


def _get_nc(apply_gb):
    key = bool(apply_gb)
    if key not in _NC_CACHE:
        _NC_CACHE[key] = _build_nc(key)
    return _NC_CACHE[key]


def _make_inputs(x, W_low, W_mid, W_high, proj_w, ln_g, ln_b):
    khbd, cs2, ICS = _host_matrices()

    W_eff = W_high[0].copy()
    W_eff[:32, :32] += W_mid[0]
    W_eff[:16, :16] += W_low[0]
    # weff[(kg,l), (k32,c256)]
    weff = np.ascontiguousarray(
        W_eff.reshape(2, 32, 64, 256).transpose(0, 2, 1, 3)
        .reshape(128, 8192).astype(ml_dtypes.bfloat16)
    )

    pjt = np.zeros((128, 512), ml_dtypes.bfloat16)
    pjt[:, :256] = proj_w.T[:128]
    pjt[:, 256:] = proj_w.T[128:]

    gb = np.stack([ln_g, ln_b]).astype(np.float32)
    consts = {"kh": khbd, "cs": cs2, "ics": ICS,
              "weff": weff, "pjt": pjt, "gb": gb}

    in_maps = []
    for b in range(B):
        m = dict(consts)
        x2d = x[b].reshape(64, 64, 256)
        for P, name in ((0, "xra"), (1, "xrb")):
            xp = x2d[:, 32 * P:32 * (P + 1), :]                   # [h, w32, c]
            m[name] = np.ascontiguousarray(
                xp.reshape(64, 2, 16, 256).transpose(1, 0, 2, 3)
                .reshape(128, 4096).astype(ml_dtypes.bfloat16)
            )
        in_maps.append(m)
    return in_maps


def kernel(x, W_low, W_mid, W_high, proj_w, ln_g, ln_b):
    x = np.ascontiguousarray(np.asarray(x, dtype=np.float32))
    W_low = np.asarray(W_low, dtype=np.float32)
    W_mid = np.asarray(W_mid, dtype=np.float32)
    W_high = np.asarray(W_high, dtype=np.float32)
    proj_w = np.asarray(proj_w, dtype=np.float32)
    ln_g = np.asarray(ln_g, dtype=np.float32)
    ln_b = np.asarray(ln_b, dtype=np.float32)

    apply_gb = not (np.all(ln_g == 1.0) and np.all(ln_b == 0.0))
    in_maps = _make_inputs(x, W_low, W_mid, W_high, proj_w, ln_g, ln_b)
    nc = _get_nc(apply_gb)
    res = run_bass_kernel_spmd(nc, in_maps, core_ids=list(range(B)))

    out = np.empty((B, N, C), np.float32)
    for b in range(B):
        yc = np.asarray(res.results[b]["y"]).astype(np.float32)
        # [r, (t2,d)] -> rows t2*128+r = n*64+h -> [h, n, d] -> [n*? ...]
        yc = yc.reshape(128, 32, 256).transpose(1, 0, 2).reshape(4096, 256)
        out[b] = yc.reshape(64, 64, 256).transpose(1, 0, 2).reshape(4096, 256)
    return out


# revision 44
# speedup vs baseline: 1.1457x; 1.1457x over previous
"""Trainium2 Bass kernel for nn_BandSplitDCTFilter.

Math: the reference's mirror-FFT DCT / band filter / inverse collapses to
    out_c = C1 (Z_c) C2^T - S1 (Z_c) S2^T,   Z_c = (A x_c A^T) .* W_eff_c
with A[k,j] = 2cos(pi k (2j+1)/128); C2/S2 carry the irfft half-spectrum
weights u_l and the 1/(4HW) scale; W_eff = pad(W_low)+pad(W_mid)+W_high
merges the three bands (they share the inverse basis under zero-padding).
Then y = x_out @ proj_w^T and LayerNorm.

Sharding: pure data-parallel, one sample per core (B=8 = 8 cores), small
weights replicated.

v10 changes over the v9 baseline (146us):
  - everything bf16 (cs2 was fp32r -> 4x matmul speedup on S5; weff and
    the output y halve their HBM traffic; host up-casts y).
  - S2/S4 use a block-diagonal 128x128 DCT basis so each stage is 8
    full-K matmuls instead of 16 half-K ones (half the instructions,
    half the PSUM drains).
  - PSUM split: 5-deep [128,512] ring for the transform stages + 3-deep
    [128,256] ring for proj tiles, so proj/LN never starves transforms.
  - proj tiles drain to SBUF (bf16) immediately after bn_stats; LN
    normalize runs per 8-tile group on gpsimd (tensor_scalar with
    per-partition scale/bias) and each group stores as soon as it is
    normalized -> short tail.
  - engine split: sync=pipe-A DMA + y stores, scalar=pipe-B DMA + S8
    casts + LN sqrt, gpsimd=consts/weff DMA + S4 Weff-mult drains +
    normalize, vector=other PSUM drains + bn stats.
  - tail interleave: s7(pipe B) / proj / LN / store proceed per Ustk
    quarter so the second pivot's DMA overlaps the projection.
"""

import os

# The Bass kernel executes on the 8 axon-tunneled NeuronCores via PJRT;
# make sure jax can see them even if the caller pinned a platform default.
os.environ.setdefault("JAX_PLATFORMS", "axon,cpu")

import numpy as np
import ml_dtypes

import bass_rust
import concourse.bass as bass
import concourse.mybir as mybir
from concourse.tile import TileContext, ScopedClock
from concourse.bass_utils import run_bass_kernel_spmd

# ---------------------------------------------------------------------------
# Workarounds: this container's walrus rejects >1 sync wait per instruction.
# ---------------------------------------------------------------------------

_wait_ctr = 0


def _split_multi_waits(nc, max_waits=1):
    global _wait_ctr
    for f in nc.m.functions:
        for bb in f.blocks:
            out = []
            dirty = False
            for ins in bb.instructions:
                si = ins.sync_info
                if si is not None and len(si.on_wait) > max_waits:
                    waits = list(si.on_wait)
                    for w in waits[:-max_waits]:
                        _wait_ctr += 1
                        nop = bass_rust.InstNoOp(name=f"I-waitsplit-{_wait_ctr}")
                        nop.engine = ins.engine
                        nop.sync_info = mybir.SyncInfo(on_wait=[w], on_update=[])
                        out.append(nop)
                    ins.sync_info = mybir.SyncInfo(
                        on_wait=waits[-max_waits:], on_update=list(si.on_update)
                    )
                    dirty = True
                out.append(ins)
            if dirty:
                bb.instructions = out


def _patched_drain_and_barrier(self, tick_clock, wait_clock):
    nc = self.nc
    probe = nc.sync.nop(nofuse=True)
    wait_clock.add_sem_waits(probe.ins, ScopedClock({None: tick_clock.global_clock}))
    si = probe.ins.sync_info
    waits = list(si.on_wait) if si is not None else []
    probe.ins.sync_info = mybir.SyncInfo(on_wait=waits[:1], on_update=[])
    name2sem = {s.name: s for s in self.sems.allocated().values()}
    for w in waits[1:]:
        nc.sync.nop(nofuse=True)._wait_ge(name2sem[w.ant_name], w.wait_value)
    nc.sync.drain()
    nc.all_engine_barrier()
    popped = nc._tile_sem_poison_stack.pop()
    assert popped is self._sem_poison
    nc.clear_and_free_semaphores(list(self.sems.allocated().values()))
    nc.all_engine_barrier()


TileContext._drain_and_barrier = _patched_drain_and_barrier

# ---------------------------------------------------------------------------

B, H, W, C = 8, 64, 64, 256
N = H * W
F32 = mybir.dt.float32
BF16 = mybir.dt.bfloat16
ALU = mybir.AluOpType
ACTF = mybir.ActivationFunctionType


def _host_matrices():
    k = np.arange(64)
    j = np.arange(64)
    ang = np.pi * k[:, None] * (2 * j[None, :] + 1) / 128.0
    A = 2.0 * np.cos(ang)
    u = np.where(k == 0, 1.0, 2.0)
    C1T = np.cos(ang)
    S1T = np.sin(ang)
    C2T = u[:, None] * np.cos(ang) / 16384.0
    S2T = u[:, None] * np.sin(ang) / 16384.0

    AT = A.T.astype(np.float32)                                   # [h, k]
    khbd = np.zeros((128, 128), np.float32)
    khbd[0:64, 0:64] = AT
    khbd[64:128, 64:128] = AT
    cs2_half = np.concatenate([C2T, S2T], axis=1)                 # [l, 128]
    cs2 = np.concatenate([cs2_half, cs2_half], axis=0)
    ICS = np.concatenate([C1T, -S1T], axis=0)
    return (khbd.astype(ml_dtypes.bfloat16),
            cs2.astype(ml_dtypes.bfloat16),
            np.ascontiguousarray(ICS.astype(ml_dtypes.bfloat16)))


_NC_CACHE = {}


def _build_nc(apply_gb):
    nc = bass.Bass(trn_type="TRN2")

    xa_d = nc.dram_tensor("xra", [128, 4096], BF16, kind="ExternalInput")
    xb_d = nc.dram_tensor("xrb", [128, 4096], BF16, kind="ExternalInput")
    kh_d = nc.dram_tensor("kh", [128, 128], BF16, kind="ExternalInput")
    cs_d = nc.dram_tensor("cs", [128, 128], BF16, kind="ExternalInput")
    ics_d = nc.dram_tensor("ics", [128, 64], BF16, kind="ExternalInput")
    wa_d = nc.dram_tensor("weffa", [128, 4096], BF16, kind="ExternalInput")
    wb_d = nc.dram_tensor("weffb", [128, 4096], BF16, kind="ExternalInput")
    pjt_d = nc.dram_tensor("pjt", [128, 512], BF16, kind="ExternalInput")
    gb_d = nc.dram_tensor("gb", [2, 256], F32, kind="ExternalInput")
    y_d = nc.dram_tensor("y", [128, 8192], BF16, kind="ExternalOutput")

    with TileContext(nc) as tc:
        with (
            tc.tile_pool(name="consts", bufs=1) as consts,
            tc.tile_pool(name="wfA", bufs=1) as wfA,
            tc.tile_pool(name="wfB", bufs=1) as wfB,
            tc.tile_pool(name="sA1", bufs=1) as sA1,
            tc.tile_pool(name="sA2", bufs=1) as sA2,
            tc.tile_pool(name="sA3", bufs=1) as sA3,
            tc.tile_pool(name="sB1", bufs=1) as sB1,
            tc.tile_pool(name="sB2", bufs=1) as sB2,
            tc.tile_pool(name="sB3", bufs=1) as sB3,
            tc.tile_pool(name="zA", bufs=1) as zA,
            tc.tile_pool(name="zB", bufs=1) as zB,
            tc.tile_pool(name="yr", bufs=1) as yr,
            tc.tile_pool(name="dramp", bufs=1, space="DRAM") as dramp,
            tc.tile_pool(name="ps", bufs=5, space="PSUM") as ps,
            tc.tile_pool(name="psy", bufs=3, space="PSUM") as psy,
            tc.tile_pool(name="small", bufs=8) as small,
        ):
            # ---- constants (gpsimd queue; x loads get sync/scalar) ----
            khbd = consts.tile([128, 128], BF16, tag="khbd")
            cs2 = consts.tile([128, 128], BF16, tag="cs2")
            ics = consts.tile([128, 64], BF16, tag="ics")
            pjt = consts.tile([128, 512], BF16, tag="pjt")
            nc.gpsimd.dma_start(out=khbd[:], in_=kh_d[:])
            nc.gpsimd.dma_start(out=cs2[:], in_=cs_d[:])
            nc.gpsimd.dma_start(out=ics[:], in_=ics_d[:])
            nc.gpsimd.dma_start(out=pjt[:], in_=pjt_d[:])
            eps = consts.tile([128, 1], F32, tag="eps")
            nc.vector.memset(eps[:], 1e-5)
            weffA = wfA.tile([128, 4096], BF16, tag="wfA")
            weffB = wfB.tile([128, 4096], BF16, tag="wfB")
            nc.gpsimd.dma_start(out=weffA[:], in_=wa_d[:])
            nc.gpsimd.dma_start(out=weffB[:], in_=wb_d[:])
            if apply_gb:
                gt = consts.tile([128, 256], F32, tag="gt")
                bt = consts.tile([128, 256], F32, tag="bt")
                gb_ap = gb_d.ap()
                g_b = bass.AP(tensor=gb_ap.tensor, offset=0, ap=[[0, 128], [1, 256]])
                b_b = bass.AP(tensor=gb_ap.tensor, offset=256, ap=[[0, 128], [1, 256]])
                nc.gpsimd.dma_start(out=gt[:], in_=g_b)
                nc.gpsimd.dma_start(out=bt[:], in_=b_b)

            cfg = {
                0: dict(x_d=xa_d, io=nc.sync, s1=sA1, s2=sA2, s3=sA3, zp=zA),
                1: dict(x_d=xb_d, io=nc.scalar, s1=sB1, s2=sB2, s3=sB3, zp=zB),
            }
            st = {0: {}, 1: {}}

            def s1_load(P):
                c = cfg[P]
                X = c["s1"].tile([128, 4096], BF16, tag=f"s{P}1")
                for q in range(4):
                    qs = slice(q * 1024, (q + 1) * 1024)
                    c["io"].dma_start(out=X[:, qs], in_=c["x_d"][:, qs])
                st[P]["X"] = X

            def s2_fh(P):
                # T1[(wh,k),(w32,c)] = blockdiag(A^T)^T @ X  (K=128 full)
                c = cfg[P]
                X = st[P]["X"]
                T1p = c["s2"].tile([128, 4096], BF16, tag=f"s{P}2")
                for j in range(8):
                    sl = slice(j * 512, (j + 1) * 512)
                    pt = ps.tile([128, 512], F32, tag="ps")
                    nc.tensor.matmul(pt[:], khbd[:], X[:, sl],
                                     start=True, stop=True)
                    eng = nc.vector.tensor_copy if j % 2 == 0 else nc.scalar.copy
                    eng(T1p[:, sl], pt[:])
                st[P]["T1p"] = T1p

            def p1_pivot(P):
                c = cfg[P]
                T1p = st[P]["T1p"]
                D1 = dramp.tile([64, 8192], BF16, tag=f"d1{P}")
                D1v = D1[:].rearrange("w (k c) -> k w c", c=128)
                T2p = c["s3"].tile([128, 4096], BF16, tag=f"s{P}3")
                c["io"].dma_start(out=D1v[0:32, 0:32, :], in_=T1p[0:32, :])
                c["io"].dma_start(out=D1v[0:32, 32:64, :], in_=T1p[64:96, :])
                for q in range(2):
                    qs = slice(q * 2048, (q + 1) * 2048)
                    c["io"].dma_start(out=T2p[0:64, qs], in_=D1[:, qs])
                c["io"].dma_start(out=D1v[32:64, 0:32, :], in_=T1p[32:64, :])
                c["io"].dma_start(out=D1v[32:64, 32:64, :], in_=T1p[96:128, :])
                for q in range(2):
                    qs = slice(q * 2048, (q + 1) * 2048)
                    c["io"].dma_start(out=T2p[64:128, qs],
                                      in_=D1[:, 4096 + q * 2048:4096 + (q + 1) * 2048])
                st[P]["T2p"] = T2p

            def s4_s5(P):
                c = cfg[P]
                T2p = st[P]["T2p"]
                weff = weffA if P == 0 else weffB
                Zp = c["zp"].tile([128, 4096], BF16, tag=f"z{P}")
                for j in range(8):
                    sl = slice(j * 512, (j + 1) * 512)
                    pt = ps.tile([128, 512], F32, tag="ps")
                    nc.tensor.matmul(pt[:], khbd[:], T2p[:, sl],
                                     start=True, stop=True)
                    nc.vector.tensor_mul(Zp[:, sl], pt[:], weff[:, sl])
                U2s = c["s3"].tile([128, 8192], BF16, tag=f"s{P}3")
                for j in range(16):
                    off = 64 * (j // 8)
                    sl = slice((j % 8) * 512, (j % 8 + 1) * 512)
                    pt = ps.tile([128, 512], F32, tag="ps")
                    nc.tensor.matmul(pt[:], cs2[off:off + 64, :],
                                     Zp[off:off + 64, sl], start=True, stop=True)
                    dsl = slice(j * 512, (j + 1) * 512)
                    eng = nc.vector.tensor_copy if j % 2 == 0 else nc.scalar.copy
                    eng(U2s[:, dsl], pt[:])
                st[P]["U2s"] = U2s

            def p2_pivot(P):
                c = cfg[P]
                U2s = st[P]["U2s"]
                D2 = dramp.tile([128, 8192], BF16, tag=f"d2{P}")
                for kh in range(2):
                    for cshalf in range(2):
                        r0 = cshalf * 64 + kh * 32
                        dst = D2[r0:r0 + 32, :].rearrange("k (n c) -> n k c", c=128)
                        c["io"].dma_start(
                            out=dst,
                            in_=U2s[cshalf * 64:(cshalf + 1) * 64,
                                    kh * 4096:(kh + 1) * 4096],
                        )
                Ustk = c["s1"].tile([128, 8192], BF16, tag=f"s{P}1")
                for q in range(4):
                    qs = slice(q * 2048, (q + 1) * 2048)
                    c["io"].dma_start(out=Ustk[:, qs], in_=D2[:, qs])
                st[P]["Ustk"] = Ustk

            def s7_alloc(P):
                c = cfg[P]
                st[P]["X01"] = c["s2"].tile([128, 4096], BF16, tag=f"s{P}2",
                                            name=f"X01_{P}")

            def s7_group(P, g):
                c = cfg[P]
                Ustk = st[P]["Ustk"]
                X01 = st[P]["X01"]
                pt = ps.tile([128, 512], F32, tag="ps")
                for nn in range(8):
                    t = 8 * g + nn
                    nc.tensor.matmul(
                        pt[:, nn * 64:(nn + 1) * 64],
                        Ustk[:, t * 128:(t + 1) * 128],
                        ics[:], start=True, stop=True,
                    )
                eng = nc.vector.tensor_copy if g % 2 == 0 else nc.scalar.copy
                eng(X01[:, g * 512:(g + 1) * 512], pt[:])

            # ---- emission: pipe A leads, pipe B staggered ----
            s1_load(0)
            s1_load(1)
            s2_fh(0)
            p1_pivot(0)
            s2_fh(1)
            s4_s5(0)
            p1_pivot(1)
            p2_pivot(0)
            s4_s5(1)
            s7_alloc(0)
            for g in range(8):
                s7_group(0, g)
            p2_pivot(1)
            s7_alloc(1)
            X01A, X01B = st[0]["X01"], st[1]["X01"]

            # ---- S8 proj + LN, interleaved with s7(pipe B) per quarter ----
            Yraw = yr.tile([128, 8192], BF16, tag="yraw")
            mvall = small.tile([128, 64], F32, tag="mvall")
            rstdall = small.tile([128, 32], F32, tag="rstdall")
            nmrall = small.tile([128, 32], F32, tag="nmrall")
            mvv = mvall[:].rearrange("p (t x) -> p t x", x=2)

            for gg in range(4):
                s7_group(1, 2 * gg)
                s7_group(1, 2 * gg + 1)
                for t2 in range(8 * gg, 8 * gg + 8):
                    pty = psy.tile([128, 256], F32, tag="psy")
                    nc.tensor.matmul(pty[:], X01A[:, t2 * 128:(t2 + 1) * 128],
                                     pjt[:, 0:256], start=True, stop=False)
                    nc.tensor.matmul(pty[:], X01B[:, t2 * 128:(t2 + 1) * 128],
                                     pjt[:, 256:512], start=False, stop=True)
                    stats = small.tile([128, 6], F32, tag="stats")
                    nc.vector.bn_stats(out=stats[:], in_=pty[:])
                    nc.vector.bn_aggr(out=mvall[:, t2 * 2:(t2 + 1) * 2], in_=stats[:])
                    nc.scalar.copy(Yraw[:, t2 * 256:(t2 + 1) * 256], pty[:])
                gs = slice(gg * 8, gg * 8 + 8)
                # std = sqrt(var + eps); rstd = 1/std; nmr = -mu*rstd
                nc.scalar.activation(out=rstdall[:, gs],
                                     in_=mvv[:, gs, 1], func=ACTF.Sqrt,
                                     bias=eps[:], scale=1.0)
                nc.vector.reciprocal(rstdall[:, gs], rstdall[:, gs])
                nc.vector.tensor_tensor(out=nmrall[:, gs], in0=mvv[:, gs, 0],
                                        in1=rstdall[:, gs], op=ALU.mult)
                nc.vector.tensor_scalar_mul(nmrall[:, gs], nmrall[:, gs], -1.0)
                for t3 in range(gg * 8, gg * 8 + 8):
                    ysl = slice(t3 * 256, (t3 + 1) * 256)
                    nc.gpsimd.tensor_scalar(
                        out=Yraw[:, ysl], in0=Yraw[:, ysl],
                        scalar1=rstdall[:, t3:t3 + 1],
                        scalar2=nmrall[:, t3:t3 + 1],
                        op0=ALU.mult, op1=ALU.add,
                    )
                    if apply_gb:
                        nc.vector.tensor_mul(Yraw[:, ysl], Yraw[:, ysl], gt[:])
                        nc.gpsimd.tensor_add(Yraw[:, ysl], Yraw[:, ysl], bt[:])
                nc.sync.dma_start(out=y_d[:, gg * 2048:(gg + 1) * 2048],
                                  in_=Yraw[:, gg * 2048:(gg + 1) * 2048])

    _split_multi_waits(nc)
    return nc


def _get_nc(apply_gb):
    key = bool(apply_gb)
    if key not in _NC_CACHE:
        _NC_CACHE[key] = _build_nc(key)
    return _NC_CACHE[key]


def _make_inputs(x, W_low, W_mid, W_high, proj_w, ln_g, ln_b):
    khbd, cs2, ICS = _host_matrices()

    W_eff = W_high[0].copy()
    W_eff[:32, :32] += W_mid[0]
    W_eff[:16, :16] += W_low[0]
    weffs = []
    for P in range(2):
        wr = W_eff[:, :, P * 128:(P + 1) * 128].transpose(1, 0, 2).reshape(64, 8192)
        weffs.append(np.ascontiguousarray(
            wr.reshape(64, 2, 4096).transpose(1, 0, 2).reshape(128, 4096)
            .astype(ml_dtypes.bfloat16)
        ))

    pjt = np.zeros((128, 512), ml_dtypes.bfloat16)
    pjt[:, :256] = proj_w.T[:128]
    pjt[:, 256:] = proj_w.T[128:]

    gb = np.stack([ln_g, ln_b]).astype(np.float32)
    consts = {"kh": khbd, "cs": cs2, "ics": ICS,
              "weffa": weffs[0], "weffb": weffs[1], "pjt": pjt, "gb": gb}

    in_maps = []
    for b in range(B):
        m = dict(consts)
        for P, name in ((0, "xra"), (1, "xrb")):
            xp = x[b].reshape(64, 64, 256)[:, :, P * 128:(P + 1) * 128]
            m[name] = np.ascontiguousarray(
                xp.reshape(64, 2, 32, 128).transpose(1, 0, 2, 3)
                .reshape(128, 4096).astype(ml_dtypes.bfloat16)
            )
        in_maps.append(m)
    return in_maps


def kernel(x, W_low, W_mid, W_high, proj_w, ln_g, ln_b):
    x = np.ascontiguousarray(np.asarray(x, dtype=np.float32))
    W_low = np.asarray(W_low, dtype=np.float32)
    W_mid = np.asarray(W_mid, dtype=np.float32)
    W_high = np.asarray(W_high, dtype=np.float32)
    proj_w = np.asarray(proj_w, dtype=np.float32)
    ln_g = np.asarray(ln_g, dtype=np.float32)
    ln_b = np.asarray(ln_b, dtype=np.float32)

    apply_gb = not (np.all(ln_g == 1.0) and np.all(ln_b == 0.0))
    in_maps = _make_inputs(x, W_low, W_mid, W_high, proj_w, ln_g, ln_b)
    nc = _get_nc(apply_gb)
    res = run_bass_kernel_spmd(nc, in_maps, core_ids=list(range(B)))

    out = np.empty((B, N, C), np.float32)
    for b in range(B):
        yc = np.asarray(res.results[b]["y"]).astype(np.float32)
        yc = yc.reshape(128, 32, 256).transpose(1, 0, 2).reshape(4096, 256)
        out[b] = yc.reshape(64, 64, 256).transpose(1, 0, 2).reshape(4096, 256)
    return out
